# revision 1
# baseline (speedup 1.0000x reference)
"""Trainium2 Bass kernel for nn_CrossAttention (16x6209x256 cross-attention).

Strategy
--------
Data-parallel over batch: 16 batches -> 8 cores x 2 batches. Each core runs an
identical Bass/Tile program on its own batch slice (pure SPMD, no collectives).

Per batch the math is
    mapped_a = a @ Wa + ba            [6209, 64]
    mapped_b = b @ Wb + bb            [256, 64]
    scores   = mapped_a @ mapped_b.T * 8
    attn     = softmax(scores, -1)
    out      = (attn @ mapped_b) @ Wc + bc

With no nonlinearity between the projections and the attention matmuls, the
small weights fold per batch (computed on device in exact fp32):
    Wfused    = 8 * Wa @ mapped_b.T               [256, 256]
    scoreBias = 8 * ba @ mapped_b.T               [256]
    Wout      = mapped_b @ Wc + 1 x bc            [256, 256]
    scores    = a @ Wfused + scoreBias
    out       = softmax(scores) @ Wout        (bias bc exact since rows sum to 1)

Precision: softmax amplifies score error by |scores| (~500 here), so the
scores matmul runs as a 3-term bf16 split (a = ahi+alo split on host,
Wfused = Whi+Wlo split on device): scores ~ ahi@Whi + alo@Whi + ahi@Wlo,
residual ~1e-4 absolute. Downstream matmuls use f32r (1.4e-4 relative,
harmless there). Weight prep runs in exact fp32.

Layout: input_a is transposed on host to [256, seq] so the contraction dim
arrives on SBUF partitions straight from DMA; output is produced transposed
[256, seq] and transposed back on host. attn is normalized in [i, j] layout
on DVE (per-partition 1/sumexp), transposed to [j, i] via PE transpose.
"""
import sys

for _p in ("/opt/trn_rl_repo",):
    if _p not in sys.path:
        sys.path.append(_p)

import numpy as np
import ml_dtypes

import concourse.bacc as bacc
import concourse.mybir as mybir
import concourse.tile as tile
from concourse.bass_utils import run_bass_kernel_spmd

F32 = mybir.dt.float32
F32R = mybir.dt.float32r
BF16 = mybir.dt.bfloat16
P = 128

N_CORES = 8
BATCHES_PER_CORE = 2
SEQ = 6209
DF = 256          # feature dim of a / b
HID = 64          # projection dim
DMA_MACRO = 2048  # rows fetched/stored per DMA instruction
CMACRO = 512      # rows per compute macro (4 subtiles of 128)


def _row_plan(n_rows):
    """[(dma_start, dma_len, [(cm_start_within_dma, cm_len), ...]), ...]"""
    plan = []
    pos = 0
    while pos < n_rows:
        d = min(DMA_MACRO, n_rows - pos)
        cms = []
        q = 0
        while q < d:
            c = min(CMACRO, d - q)
            cms.append((q, c))
            q += c
        plan.append((pos, d, cms))
        pos += d
    return plan


def build_program(seq=SEQ, batches=BATCHES_PER_CORE, use_ba=False):
    nc = bacc.Bacc("TRN2", target_bir_lowering=False, debug=False)

    a_hl = nc.dram_tensor("a_hl", [batches, 2 * DF, seq], BF16, kind="ExternalInput")
    b_t = nc.dram_tensor("b_t", [batches, DF, DF], F32, kind="ExternalInput")
    wat = nc.dram_tensor("wat", [HID, DF], F32, kind="ExternalInput")
    wb = nc.dram_tensor("wb", [DF, HID], F32, kind="ExternalInput")
    wc = nc.dram_tensor("wc", [HID, DF], F32, kind="ExternalInput")
    ba_d = nc.dram_tensor("ba_d", [HID, 1], F32, kind="ExternalInput")
    bb_d = nc.dram_tensor("bb_d", [HID, 1], F32, kind="ExternalInput")
    bc_d = nc.dram_tensor("bc_d", [1, DF], F32, kind="ExternalInput")
    eye_d = nc.dram_tensor("eye_d", [P, P], F32, kind="ExternalInput")
    ones_d = nc.dram_tensor("ones_d", [1, P], F32, kind="ExternalInput")
    out_t = nc.dram_tensor("out_t", [batches, DF, seq], F32, kind="ExternalOutput")

    Exp = mybir.ActivationFunctionType.Exp
    Copy = mybir.ActivationFunctionType.Copy
    Ident = mybir.ActivationFunctionType.Identity

    with tile.TileContext(nc) as tc:
        with (
            tc.tile_pool(name="const", bufs=1) as cpool,
            tc.tile_pool(name="wpool", bufs=2) as wpool,
            tc.tile_pool(name="apool", bufs=3) as apool,
            tc.tile_pool(name="mpool", bufs=2) as mpool,
            tc.tile_pool(name="opool", bufs=3) as opool,
            tc.tile_pool(name="pp", bufs=1, space="PSUM") as pp,
        ):
            # ---- per-core constants ----
            eye_sb = cpool.tile([P, P], F32)
            nc.sync.dma_start(eye_sb[:], eye_d[:])
            wat_sb = cpool.tile([HID, DF], F32)
            nc.sync.dma_start(wat_sb[:], wat[:])
            wb_sb = cpool.tile([P, 2, HID], F32)
            nc.sync.dma_start(wb_sb[:], wb[:].rearrange("(k p) h -> p k h", p=P))
            wc_sb = cpool.tile([HID, DF], F32)
            nc.sync.dma_start(wc_sb[:], wc[:])
            ba_sb = cpool.tile([HID, 1], F32)
            nc.sync.dma_start(ba_sb[:], ba_d[:])
            bb_sb = cpool.tile([HID, 1], F32)
            nc.sync.dma_start(bb_sb[:], bb_d[:])
            bc_sb = cpool.tile([1, DF], F32)
            nc.sync.dma_start(bc_sb[:], bc_d[:])
            ones_sb = cpool.tile([1, P], F32)
            nc.sync.dma_start(ones_sb[:], ones_d[:])

            for b in range(batches):
                # ---- per-batch fused weights (exact fp32 matmuls) ----
                bT_sb = wpool.tile([P, 2, DF], F32)
                nc.sync.dma_start(bT_sb[:], b_t[b].rearrange("(k p) j -> p k j", p=P))

                ps_mb = pp.tile([HID, DF], F32, tag="fin0")
                for k in range(2):
                    nc.tensor.matmul(
                        ps_mb[:],
                        wb_sb[:, k, :],
                        bT_sb[:, k, :],
                        start=(k == 0), stop=(k == 1),
                    )
                mapped_bT = wpool.tile([HID, DF], F32)
                nc.scalar.activation(mapped_bT[:], ps_mb[:], Ident, bias=bb_sb[:])

                # Wfused, split hi/lo into bf16 (scale 8 folded in)
                whi_sb = wpool.tile([P, 2, DF], BF16)
                wlo_sb = wpool.tile([P, 2, DF], BF16)
                for c in range(2):
                    ps_wf = pp.tile([P, DF], F32, tag="fin0")
                    nc.tensor.matmul(
                        ps_wf[:],
                        wat_sb[:, c * P:(c + 1) * P],
                        mapped_bT[:],
                        start=True, stop=True,
                    )
                    nc.scalar.activation(whi_sb[:, c, :], ps_wf[:], Copy, scale=8.0)
                    # wlo = 8*wf - whi (rounded to bf16)
                    nc.vector.scalar_tensor_tensor(
                        wlo_sb[:, c, :],
                        ps_wf[:],
                        8.0,
                        whi_sb[:, c, :],
                        op0=mybir.AluOpType.mult,
                        op1=mybir.AluOpType.subtract,
                    )

                if use_ba:
                    ps_sbias = pp.tile([1, DF], F32, tag="fin0")
                    nc.tensor.matmul(
                        ps_sbias[:],
                        ba_sb[:],
                        mapped_bT[:],
                        start=True, stop=True,
                    )
                    sbias_sb = wpool.tile([1, DF], F32)
                    nc.scalar.activation(sbias_sb[:], ps_sbias[:], Copy, scale=8.0)

                wo_sb = wpool.tile([P, 2, DF], F32R)
                for c in range(2):
                    ps_wo = pp.tile([P, DF], F32, tag="fin0")
                    nc.tensor.matmul(
                        ps_wo[:],
                        mapped_bT[:, c * P:(c + 1) * P],
                        wc_sb[:],
                        start=True, stop=False,
                    )
                    nc.tensor.matmul(
                        ps_wo[:],
                        ones_sb[:],
                        bc_sb[:],
                        start=False, stop=True,
                    )
                    nc.vector.tensor_copy(wo_sb[:, c, :], ps_wo[:])

                # ---- main loop ----
                for d0, dlen, cms in _row_plan(seq):
                    aT_sb = apool.tile([P, 4, DMA_MACRO], BF16, tag="aT")
                    nc.sync.dma_start(
                        aT_sb[:, :, :dlen],
                        a_hl[b][:, d0:d0 + dlen].rearrange(
                            "(k p) i -> p k i", p=P),
                    )
                    outT_sb = opool.tile([P, 2, DMA_MACRO], F32, tag="outT")

                    for mo, R in cms:
                        subs = [(o, min(P, R - o)) for o in range(0, R, P)]
                        ns = len(subs)

                        scores_ps = pp.tile([P, 4 * DF], F32, tag="scores", bufs=2)
                        for s, (io, r) in enumerate(subs):
                            c0 = s * DF
                            terms = []
                            for k in range(2):
                                ah = aT_sb[:, k, mo + io:mo + io + r]
                                al = aT_sb[:, 2 + k, mo + io:mo + io + r]
                                terms += [
                                    (ah, whi_sb[:, k, :]),
                                    (al, whi_sb[:, k, :]),
                                    (ah, wlo_sb[:, k, :]),
                                ]
                            for t, (lhs, rhs) in enumerate(terms):
                                nc.tensor.matmul(
                                    scores_ps[:r, c0:c0 + DF],
                                    lhs,
                                    rhs,
                                    start=(t == 0),
                                    stop=(t == len(terms) - 1) and not use_ba,
                                )
                            if use_ba:
                                nc.tensor.matmul(
                                    scores_ps[:r, c0:c0 + DF],
                                    ones_sb[:, :r],
                                    sbias_sb[:],
                                    start=False, stop=True,
                                )

                        rmax = max(r for _, r in subs)
                        negmax = mpool.tile([P, 4], F32, tag="negmax")
                        if all(r == rmax for _, r in subs):
                            nc.vector.tensor_reduce(
                                negmax[:rmax, :ns],
                                scores_ps[:rmax, :ns * DF].rearrange(
                                    "p (s j) -> p s j", s=ns),
                                axis=mybir.AxisListType.X,
                                op=mybir.AluOpType.max,
                                negate=True,
                            )
                        else:
                            for s, (io, r) in enumerate(subs):
                                nc.vector.tensor_reduce(
                                    negmax[:r, s:s + 1],
                                    scores_ps[:r, s * DF:(s + 1) * DF],
                                    axis=mybir.AxisListType.X,
                                    op=mybir.AluOpType.max,
                                    negate=True,
                                )

                        attn_sb = mpool.tile([P, 4 * DF], F32, tag="attn")
                        attn_n = mpool.tile([P, 4 * DF], F32, tag="attn_n")
                        sumexp = mpool.tile([P, 4], F32, tag="sumexp")
                        for s, (io, r) in enumerate(subs):
                            c0 = s * DF
                            nc.scalar.activation(
                                attn_sb[:r, c0:c0 + DF],
                                scores_ps[:r, c0:c0 + DF],
                                Exp,
                                bias=negmax[:r, s:s + 1],
                                accum_out=sumexp[:r, s:s + 1],
                            )
                        recip = mpool.tile([P, 4], F32, tag="recip")
                        if all(r == rmax for _, r in subs):
                            nc.vector.reciprocal(recip[:rmax, :ns], sumexp[:rmax, :ns])
                        else:
                            for s, (io, r) in enumerate(subs):
                                nc.vector.reciprocal(
                                    recip[:r, s:s + 1], sumexp[:r, s:s + 1])
                        for s, (io, r) in enumerate(subs):
                            c0 = s * DF
                            nc.vector.tensor_scalar_mul(
                                attn_n[:r, c0:c0 + DF],
                                attn_sb[:r, c0:c0 + DF],
                                recip[:r, s:s + 1],
                            )

                        aT0_ps = pp.tile([P, CMACRO], F32, tag="attnT0")
                        aT1_ps = pp.tile([P, CMACRO], F32, tag="attnT1")
                        for s, (io, r) in enumerate(subs):
                            c0 = s * DF
                            for jh, dst in ((0, aT0_ps), (1, aT1_ps)):
                                o_ap = dst[:, io:io + r]
                                i_ap = attn_n[:r, c0 + jh * P:c0 + (jh + 1) * P]
                                e_ap = eye_sb[:r, :r]
                                if r % 2:
                                    # f32r transpose needs an even moving dim
                                    o_ap = o_ap.bitcast(F32)
                                    i_ap = i_ap.bitcast(F32)
                                    e_ap = e_ap.bitcast(F32)
                                nc.tensor.transpose(o_ap, i_ap, e_ap)
                        attnT0 = mpool.tile([P, CMACRO], F32R, tag="attnT0sb")
                        attnT1 = mpool.tile([P, CMACRO], F32R, tag="attnT1sb")
                        nc.scalar.copy(attnT0[:, :R], aT0_ps[:, :R])
                        nc.vector.tensor_copy(attnT1[:, :R], aT1_ps[:, :R])

                        # final: outT[fo, i] = sum_j Wout[j, fo] attnT[j, i]
                        for c in range(2):
                            ps_fin = pp.tile([P, CMACRO], F32, tag=f"fin{c}")
                            for k, aTk in enumerate((attnT0, attnT1)):
                                # f32r needs an even moving dim; odd tails
                                # fall back to plain fp32 (tiny anyway)
                                if R % 2 == 0:
                                    lhs, rhs = (wo_sb[:, k, c * P:(c + 1) * P],
                                                aTk[:, :R])
                                else:
                                    lhs = wo_sb[:, k, c * P:(c + 1) * P].bitcast(F32)
                                    rhs = aTk[:, :R].bitcast(F32)
                                nc.tensor.matmul(
                                    ps_fin[:, :R],
                                    lhs,
                                    rhs,
                                    start=(k == 0), stop=(k == 1),
                                )
                            if c == 0:
                                nc.vector.tensor_copy(
                                    outT_sb[:, c, mo:mo + R], ps_fin[:, :R])
                            else:
                                nc.scalar.copy(
                                    outT_sb[:, c, mo:mo + R], ps_fin[:, :R])

                    nc.sync.dma_start(
                        out_t[b][:, d0:d0 + dlen].rearrange("(c p) i -> p c i", p=P),
                        outT_sb[:, :, :dlen],
                    )

    nc.compile()
    return nc


_PROGRAM_CACHE = {}


def _get_program(seq=SEQ, batches=BATCHES_PER_CORE, use_ba=False):
    key = (seq, batches, use_ba)
    if key not in _PROGRAM_CACHE:
        _PROGRAM_CACHE[key] = build_program(seq, batches, use_ba)
    return _PROGRAM_CACHE[key]


def make_in_maps(input_a, input_b, Wa, ba, Wb, bb, Wc, bc,
                 n_cores=N_CORES, batches=BATCHES_PER_CORE):
    input_a = np.asarray(input_a, dtype=np.float32)
    input_b = np.asarray(input_b, dtype=np.float32)
    a_t = np.ascontiguousarray(input_a.transpose(0, 2, 1))      # [B, DF, seq]
    a_hi = a_t.astype(ml_dtypes.bfloat16)
    a_lo = (a_t - a_hi.astype(np.float32)).astype(ml_dtypes.bfloat16)
    # rows 0..DF-1 = hi, DF..2DF-1 = lo  -> [B, 2*DF, seq]
    a_hl = np.ascontiguousarray(np.concatenate([a_hi, a_lo], axis=1))
    b_t = np.ascontiguousarray(input_b.transpose(0, 2, 1))
    shared = {
        "wat": np.ascontiguousarray(np.asarray(Wa, np.float32).T),
        "wb": np.ascontiguousarray(np.asarray(Wb, np.float32)),
        "wc": np.ascontiguousarray(np.asarray(Wc, np.float32)),
        "ba_d": np.asarray(ba, np.float32).reshape(HID, 1).copy(),
        "bb_d": np.asarray(bb, np.float32).reshape(HID, 1).copy(),
        "bc_d": np.asarray(bc, np.float32).reshape(1, DF).copy(),
        "eye_d": np.eye(P, dtype=np.float32),
        "ones_d": np.ones((1, P), dtype=np.float32),
    }
    in_maps = []
    for c in range(n_cores):
        lo, hi = c * batches, (c + 1) * batches
        in_maps.append({
            "a_hl": np.ascontiguousarray(a_hl[lo:hi]),
            "b_t": np.ascontiguousarray(b_t[lo:hi]),
            **shared,
        })
    return in_maps


def kernel(input_a, input_b, Wa, ba, Wb, bb, Wc, bc):
    use_ba = bool(np.any(np.asarray(ba)))
    nc = _get_program(use_ba=use_ba)
    in_maps = make_in_maps(input_a, input_b, Wa, ba, Wb, bb, Wc, bc)
    res = run_bass_kernel_spmd(nc, in_maps, core_ids=list(range(N_CORES)))
    outs = np.concatenate([r["out_t"] for r in res.results], axis=0)
    return np.ascontiguousarray(outs.transpose(0, 2, 1))



# revision 2
# speedup vs baseline: 1.1088x; 1.1088x over previous
"""Trainium2 Bass kernel for nn_CrossAttention (16x6209x256 cross-attention).

Strategy (v2, "T16")
--------------------
Data-parallel over batch: 16 batches -> 8 cores x 2 batches, pure SPMD.

Per batch:
    mapped_a = a @ Wa + ba            [seq, 64]
    mapped_b = b @ Wb + bb            [256, 64]
    scores   = mapped_a @ mapped_b.T * 8
    attn     = softmax(scores, -1)
    out      = (attn @ mapped_b) @ Wc + bc

Rather than folding Wa @ mapped_b.T into a rank-64 [256, 256] matrix and
paying a K=256 matmul per score tile (what v1 did), exploit the rank-64
structure directly: stage 1 computes mapped_a^T [64, seq] with the cheap
i-moving orientation (weights stationary), stage 2 computes each [128, 256]
score tile from a K=128 stationary [ma_hi; ma_lo] stack.

Precision: fp16 matmuls accumulate exactly in fp32 PSUM, so hi/lo fp16
splits give ~22-bit operands:
  stage 1:  [a_hi; a_lo] @ Wa16  +  a_hi @ (Wa - Wa16)      (host splits a)
  stage 2:  [ma_hi; ma_lo] @ (mb16 + dmb16)                 (device splits ma)
Scores come out accurate to ~1e-3 absolute; the fp16 softmax/output path
contributes ~8e-4 relative overall (measured in numpy simulation).

Layout tricks:
  - stage 1 is column-tiled: chunk pairs write the two PSUM partition
    halves from [128, 64] stationaries, running concurrently on the PE.
  - stage 2 reuses ONE stationary for both its matmuls (mb16 and dmb16
    moving), so there is a single weight load per 128 rows.
  - attn is normalized on DVE pre-transpose (PE transpose is pure data
    movement; the identity operand is ignored).
  - everything downstream of exp is fp16: transposes cost 1 cycle/row,
    DVE copies run at 2x, and the output DMA is half-size.
seq is host-padded to 6272 (49 x 128) so no odd-size tiles exist.
"""
import sys

for _p in ("/opt/trn_rl_repo",):
    if _p not in sys.path:
        sys.path.append(_p)

import numpy as np
import ml_dtypes

import concourse.bacc as bacc
import concourse.mybir as mybir
import concourse.tile as tile
from concourse.bass_utils import run_bass_kernel_spmd

F32 = mybir.dt.float32
F16 = mybir.dt.float16
P = 128

N_CORES = 8
BATCHES_PER_CORE = 2
SEQ = 6209
SEQP = 6272            # 49 * 128
DF = 256
HID = 64
DMA_MACRO = 2048
CHUNK = 512


def _chunks(w):
    """[(offset, width)] chunks of 512 then tail."""
    out = []
    pos = 0
    while pos < w:
        c = min(CHUNK, w - pos)
        out.append((pos, c))
        pos += c
    return out


def _blocks(seqp):
    out = []
    pos = 0
    while pos < seqp:
        w = min(DMA_MACRO, seqp - pos)
        out.append((pos, w))
        pos += w
    return out


def build_program(seqp=SEQP, batches=BATCHES_PER_CORE, use_ba=False):
    nc = bacc.Bacc("TRN2", target_bir_lowering=False, debug=False)

    a_hl = nc.dram_tensor("a_hl", [batches, 2 * DF, seqp], F16, kind="ExternalInput")
    b_t = nc.dram_tensor("b_t", [batches, DF, DF], F32, kind="ExternalInput")
    wa16_d = nc.dram_tensor("wa16_d", [DF, HID], F16, kind="ExternalInput")
    dwa_d = nc.dram_tensor("dwa_d", [DF, HID], F16, kind="ExternalInput")
    wb_d = nc.dram_tensor("wb_d", [DF, HID], F32, kind="ExternalInput")
    wc_d = nc.dram_tensor("wc_d", [HID, DF], F32, kind="ExternalInput")
    ba_d = nc.dram_tensor("ba_d", [HID, 1], F32, kind="ExternalInput")
    bb_d = nc.dram_tensor("bb_d", [HID, 1], F32, kind="ExternalInput")
    bb8_d = nc.dram_tensor("bb8_d", [HID, 1], F32, kind="ExternalInput")
    bc_d = nc.dram_tensor("bc_d", [1, DF], F32, kind="ExternalInput")
    eye_d = nc.dram_tensor("eye_d", [P, P], F16, kind="ExternalInput")
    ones_d = nc.dram_tensor("ones_d", [1, P], F32, kind="ExternalInput")
    out_t = nc.dram_tensor("out_t", [batches, DF, seqp], F16, kind="ExternalOutput")

    Exp = mybir.ActivationFunctionType.Exp
    Copy = mybir.ActivationFunctionType.Copy
    Ident = mybir.ActivationFunctionType.Identity

    with tile.TileContext(nc) as tc:
        with (
            tc.tile_pool(name="const", bufs=1) as cpool,
            tc.tile_pool(name="wpool", bufs=2) as wpool,
            tc.tile_pool(name="apool", bufs=2) as apool,
            tc.tile_pool(name="spool", bufs=4) as spool,
            tc.tile_pool(name="mpool", bufs=4) as mpool,
            tc.tile_pool(name="opool", bufs=2) as opool,
            tc.tile_pool(name="pp", bufs=1, space="PSUM") as pp,
        ):
            # ---- per-core constants ----
            eye_sb = cpool.tile([P, P], F16)
            nc.sync.dma_start(eye_sb[:], eye_d[:])
            wa16_sb = cpool.tile([P, 2, HID], F16)
            nc.sync.dma_start(wa16_sb[:], wa16_d[:].rearrange("(k p) h -> p k h", p=P))
            dwa_sb = cpool.tile([P, 2, HID], F16)
            nc.sync.dma_start(dwa_sb[:], dwa_d[:].rearrange("(k p) h -> p k h", p=P))
            wb_sb = cpool.tile([P, 2, HID], F32)
            nc.sync.dma_start(wb_sb[:], wb_d[:].rearrange("(k p) h -> p k h", p=P))
            wc_sb = cpool.tile([HID, DF], F32)
            nc.sync.dma_start(wc_sb[:], wc_d[:])
            ba_sb = cpool.tile([HID, 1], F32)
            nc.sync.dma_start(ba_sb[:], ba_d[:])
            bb_sb = cpool.tile([HID, 1], F32)
            nc.sync.dma_start(bb_sb[:], bb_d[:])
            bb8_sb = cpool.tile([HID, 1], F32)
            nc.sync.dma_start(bb8_sb[:], bb8_d[:])
            bc_sb = cpool.tile([1, DF], F32)
            nc.sync.dma_start(bc_sb[:], bc_d[:])
            ones_sb = cpool.tile([1, P], F32)
            nc.sync.dma_start(ones_sb[:], ones_d[:])

            for b in range(batches):
                # ---- per-batch prep (exact fp32) ----
                bT_sb = wpool.tile([P, 2, DF], F32)
                nc.sync.dma_start(bT_sb[:], b_t[b].rearrange("(k p) j -> p k j", p=P))

                ps_mb = pp.tile([HID, DF], F32, tag="ma", bufs=2)
                for k in range(2):
                    nc.tensor.matmul(
                        ps_mb[:], wb_sb[:, k, :], bT_sb[:, k, :],
                        start=(k == 0), stop=(k == 1),
                    )
                # mb16 + dmb16, each duplicated on both partition halves
                mb16_sb = wpool.tile([P, DF], F16)     # rows 0-63 and 64-127 both = mb16
                dmb16_sb = wpool.tile([P, DF], F16)
                nc.scalar.activation(mb16_sb[:HID, :], ps_mb[:], Ident,
                                     bias=bb8_sb[:], scale=8.0)
                nc.vector.scalar_tensor_tensor(
                    dmb16_sb[:HID, :], ps_mb[:], 8.0, mb16_sb[:HID, :],
                    op0=mybir.AluOpType.mult, op1=mybir.AluOpType.subtract,
                )
                nc.sync.dma_start(mb16_sb[HID:, :], mb16_sb[:HID, :])
                nc.sync.dma_start(dmb16_sb[HID:, :], dmb16_sb[:HID, :])

                # score bias row (only when ba != 0): sbias = 8*ba^T mapped_b^T
                if use_ba:
                    mb32_sb = wpool.tile([HID, DF], F32)
                    nc.scalar.activation(mb32_sb[:], ps_mb[:], Ident, bias=bb_sb[:])
                    ps_sb = pp.tile([1, DF], F32, tag="sc", bufs=2)
                    nc.tensor.matmul(ps_sb[:], ba_sb[:], mb32_sb[:],
                                     start=True, stop=True)
                    sbias_sb = wpool.tile([1, DF], F32)
                    nc.scalar.activation(sbias_sb[:], ps_sb[:], Copy, scale=8.0)
                else:
                    mb32_sb = wpool.tile([HID, DF], F32)
                    nc.scalar.activation(mb32_sb[:], ps_mb[:], Ident, bias=bb_sb[:])

                # Wout = mapped_b @ Wc + 1 x bc   -> fp16 stationaries
                wout_sb = wpool.tile([P, 2, DF], F16)
                for jh in range(2):
                    ps_wo = pp.tile([P, DF], F32, tag=f"fin{jh}")
                    nc.tensor.matmul(ps_wo[:], mb32_sb[:, jh * P:(jh + 1) * P],
                                     wc_sb[:], start=True, stop=False)
                    nc.tensor.matmul(ps_wo[:], ones_sb[:], bc_sb[:],
                                     start=False, stop=True)
                    if jh == 0:
                        nc.vector.tensor_copy(wout_sb[:, 0, :], ps_wo[:])
                    else:
                        nc.scalar.copy(wout_sb[:, 1, :], ps_wo[:])

                # ---- main loop ----
                for d0, W in _blocks(seqp):
                    aT = apool.tile([P, 4, DMA_MACRO], F16, tag="aT")
                    nc.sync.dma_start(
                        aT[:, :, :W],
                        a_hl[b][:, d0:d0 + W].rearrange("(g p) i -> p g i", p=P),
                    )
                    outT = opool.tile([P, 2, DMA_MACRO], F16, tag="outT")

                    # pair up full 512-chunks for column tiling
                    chs = _chunks(W)
                    groups = []
                    i = 0
                    while i < len(chs):
                        if i + 1 < len(chs) and chs[i][1] == CHUNK and chs[i + 1][1] == CHUNK:
                            groups.append((chs[i], chs[i + 1]))
                            i += 2
                        else:
                            groups.append((chs[i],))
                            i += 1

                    for grp in groups:
                        cw = grp[0][1]
                        # stage 1: mapped_a^T into psum halves (column-tiled)
                        ps_ma = pp.tile([P, CHUNK], F32, tag="ma", bufs=2)
                        for half, (coff, w) in zip((0, HID), grp):
                            terms = [(0, wa16_sb, 0), (1, wa16_sb, 1),
                                     (2, wa16_sb, 0), (3, wa16_sb, 1),
                                     (0, dwa_sb, 0), (1, dwa_sb, 1)]
                            for t, (g, wsb, k) in enumerate(terms):
                                nc.tensor.matmul(
                                    ps_ma[half:half + HID, :w],
                                    wsb[:, k, :],
                                    aT[:, g, coff:coff + w],
                                    start=(t == 0), stop=(t == len(terms) - 1),
                                )

                        # build [ma_hi; ma_lo] stacks (one per chunk)
                        stks = []
                        lotmp = mpool.tile([P, CHUNK], F16, tag="lotmp")
                        for half, (coff, w) in zip((0, HID), grp):
                            stk = spool.tile([P, CHUNK], F16, tag="stk")
                            oh = HID - half  # opposite half
                            if half == 0:
                                nc.scalar.copy(stk[:HID, :w], ps_ma[:HID, :w])
                            else:
                                nc.scalar.copy(stk[HID:, :w], ps_ma[HID:, :w])
                            nc.vector.scalar_tensor_tensor(
                                lotmp[half:half + HID, :w],
                                ps_ma[half:half + HID, :w],
                                1.0,
                                stk[half:half + HID, :w],
                                op0=mybir.AluOpType.mult,
                                op1=mybir.AluOpType.subtract,
                            )
                            nc.sync.dma_start(stk[oh:oh + HID, :w],
                                              lotmp[half:half + HID, :w])
                            stks.append(stk)

                        # stage 2 + softmax + transpose + output, per chunk
                        for stk, (coff, w) in zip(stks, grp):
                            aT_ps = pp.tile([P, 2, CHUNK], F16, tag="aTp", bufs=2)
                            for s0 in range(0, w, P):
                                sc_ps = pp.tile([P, DF], F32, tag="sc", bufs=2)
                                nc.tensor.matmul(sc_ps[:], stk[:, s0:s0 + P],
                                                 mb16_sb[:], start=True, stop=False)
                                nc.tensor.matmul(sc_ps[:], stk[:, s0:s0 + P],
                                                 dmb16_sb[:], start=False,
                                                 stop=not use_ba)
                                if use_ba:
                                    nc.tensor.matmul(sc_ps[:], ones_sb[:],
                                                     sbias_sb[:], start=False,
                                                     stop=True)
                                negmax = mpool.tile([P, 1], F32, tag="nm")
                                nc.vector.tensor_reduce(
                                    negmax[:], sc_ps[:],
                                    axis=mybir.AxisListType.X,
                                    op=mybir.AluOpType.max, negate=True,
                                )
                                attn = mpool.tile([P, DF], F16, tag="attn")
                                sumexp = mpool.tile([P, 1], F32, tag="se")
                                nc.scalar.activation(
                                    attn[:], sc_ps[:], Exp,
                                    bias=negmax[:], accum_out=sumexp[:],
                                )
                                recip = mpool.tile([P, 1], F32, tag="rc")
                                nc.vector.reciprocal(recip[:], sumexp[:])
                                attn_n = mpool.tile([P, DF], F16, tag="attn_n")
                                nc.vector.tensor_scalar_mul(
                                    attn_n[:], attn[:], recip[:])
                                for jh in range(2):
                                    nc.tensor.transpose(
                                        aT_ps[:, jh, s0:s0 + P],
                                        attn_n[:, jh * P:(jh + 1) * P],
                                        eye_sb[:],
                                    )
                            attnT = mpool.tile([P, 2, CHUNK], F16, tag="attnT")
                            nc.vector.tensor_copy(attnT[:, 0, :w], aT_ps[:, 0, :w])
                            nc.scalar.copy(attnT[:, 1, :w], aT_ps[:, 1, :w])

                            for fh in range(2):
                                ps_f = pp.tile([P, CHUNK], F32, tag=f"fin{fh}")
                                for jh in range(2):
                                    nc.tensor.matmul(
                                        ps_f[:, :w],
                                        wout_sb[:, jh, fh * P:(fh + 1) * P],
                                        attnT[:, jh, :w],
                                        start=(jh == 0), stop=(jh == 1),
                                    )
                                if fh == 0:
                                    nc.vector.tensor_copy(
                                        outT[:, 0, coff:coff + w], ps_f[:, :w])
                                else:
                                    nc.scalar.copy(
                                        outT[:, 1, coff:coff + w], ps_f[:, :w])

                    nc.sync.dma_start(
                        out_t[b][:, d0:d0 + W].rearrange("(c p) i -> p c i", p=P),
                        outT[:, :, :W],
                    )

    nc.compile()
    return nc


_PROGRAM_CACHE = {}


def _get_program(seqp=SEQP, batches=BATCHES_PER_CORE, use_ba=False):
    key = (seqp, batches, use_ba)
    if key not in _PROGRAM_CACHE:
        _PROGRAM_CACHE[key] = build_program(seqp, batches, use_ba)
    return _PROGRAM_CACHE[key]


def make_in_maps(input_a, input_b, Wa, ba, Wb, bb, Wc, bc,
                 n_cores=N_CORES, batches=BATCHES_PER_CORE, seqp=SEQP):
    input_a = np.asarray(input_a, dtype=np.float32)
    input_b = np.asarray(input_b, dtype=np.float32)
    nb, seq, _ = input_a.shape
    a_t = input_a.transpose(0, 2, 1)                         # [B, DF, seq]
    if seqp > seq:
        a_t = np.concatenate(
            [a_t, np.zeros((nb, DF, seqp - seq), np.float32)], axis=2)
    a_hi = a_t.astype(np.float16)
    a_lo = (a_t - a_hi.astype(np.float32)).astype(np.float16)
    a_hl = np.ascontiguousarray(np.concatenate([a_hi, a_lo], axis=1))
    b_t = np.ascontiguousarray(input_b.transpose(0, 2, 1))

    Wa = np.asarray(Wa, np.float32)
    wa16 = Wa.astype(np.float16)
    dwa = (Wa - wa16.astype(np.float32)).astype(np.float16)
    bb_ = np.asarray(bb, np.float32).reshape(HID, 1)
    shared = {
        "wa16_d": np.ascontiguousarray(wa16),
        "dwa_d": np.ascontiguousarray(dwa),
        "wb_d": np.ascontiguousarray(np.asarray(Wb, np.float32)),
        "wc_d": np.ascontiguousarray(np.asarray(Wc, np.float32)),
        "ba_d": np.asarray(ba, np.float32).reshape(HID, 1).copy(),
        "bb_d": bb_.copy(),
        "bb8_d": (8.0 * bb_).copy(),
        "bc_d": np.asarray(bc, np.float32).reshape(1, DF).copy(),
        "eye_d": np.eye(P, dtype=np.float16),
        "ones_d": np.ones((1, P), dtype=np.float32),
    }
    in_maps = []
    for c in range(n_cores):
        lo, hi = c * batches, (c + 1) * batches
        in_maps.append({
            "a_hl": np.ascontiguousarray(a_hl[lo:hi]),
            "b_t": np.ascontiguousarray(b_t[lo:hi]),
            **shared,
        })
    return in_maps


def postprocess(res, seq=SEQ):
    outs = np.concatenate([r["out_t"] for r in res.results], axis=0)
    return np.ascontiguousarray(
        outs[:, :, :seq].transpose(0, 2, 1).astype(np.float32))


def kernel(input_a, input_b, Wa, ba, Wb, bb, Wc, bc):
    use_ba = bool(np.any(np.asarray(ba)))
    nc = _get_program(use_ba=use_ba)
    in_maps = make_in_maps(input_a, input_b, Wa, ba, Wb, bb, Wc, bc)
    res = run_bass_kernel_spmd(nc, in_maps, core_ids=list(range(N_CORES)))
    return postprocess(res, seq=np.asarray(input_a).shape[1])
